# revision 31
# baseline (speedup 1.0000x reference)
"""Causal self-attention (B=2, T=2048, C=1024, H=16) on 8 Trainium2 cores.

Sharding: tensor-parallel over heads (2 heads/core). Each core computes
q/k/v for its heads, causal attention, and its slice of the c_proj
contraction; the host sums the 8 partial projection outputs and adds
b_proj.

Device-side layout keeps activations transposed ([feat, tok]) so no
transposes of x/q/k are needed; v is transposed on-chip via DMA-xbar.
Softmax runs over the partition axis of S^T: the denominator comes for
free from a ones-column appended to v in the P@V matmul.

v2 engine budget: ACT runs only exp (+qkv bias evac); DVE does all
PSUM evacuation, the causal mask (mul by a precomputed triangle), and
approx reciprocals; GpSimd only does the tiny normalize hops; both
heads' S matmuls row-pack into one 2-bank PSUM tile so exp covers both
heads in (mostly) one instruction.
"""

import sys

try:
    import concourse  # noqa: F401
except ImportError:
    sys.path.insert(0, "/opt/trn_rl_repo")

import numpy as np
import ml_dtypes

import concourse.bacc as bacc
import concourse.mybir as mybir
import concourse.tile as tile
from concourse import bass_utils

B, T, C, H, NCORES = 2, 2048, 1024, 16, 8
BT = B * T                  # 4096 tokens total
HPC = H // NCORES           # 2 heads per core
D = C // H                  # 64 head dim
CS = HPC * D                # 128 per-core feature slice
QB = 512                    # q block (free dim per matmul)
KT = 128                    # k tile (partition dim of S^T)
NB = T // QB                # 4 q-blocks per batch
NKT = T // KT               # 16 k-tiles per batch
NCT = C // 128              # 8 contraction tiles over C
BF16 = mybir.dt.bfloat16
F32 = mybir.dt.float32
SCALE = 1.0 / np.sqrt(D)

_built = {}

# ---- tuning knobs (defaults = shipping config) ----
QKV_EVAC = "scalar"      # "scalar" (ACT) | "vector" (DVE)
PSO_BUFS = 4             # psO bufs (4 = two blocks' accumulators live)
PSS_MODE = "perhead"     # "shared" [128,1024]x2 | "perhead" [128,512]xN
PSS_BUFS = 2             # 0 = auto (shared:2 / perhead:4)
PHASES = "all"           # "all" | "noattn" | "attnonly" (timing ablations)
TRP_MODE = "pe"          # "xbar" DMA transpose | "pe" PE transpose+DVE copy
SCHEDULE = "dual"        # "wave" | "dual" (interleave both batches' blocks)
OUT_SPLIT = False        # split output stores across both HWDGE rings


def _build(repeat=1):
    key = ("nc", repeat, QKV_EVAC, PSO_BUFS, PSS_MODE, PSS_BUFS, PHASES,
           TRP_MODE, SCHEDULE, OUT_SPLIT)
    if key in _built:
        return _built[key]

    nc = bacc.Bacc("TRN2", target_bir_lowering=False, debug=False,
                   num_devices=NCORES)
    xT = nc.dram_tensor("xT", [C, BT], BF16, kind="ExternalInput")
    wqkv = nc.dram_tensor("wqkv", [C, 3 * CS], BF16, kind="ExternalInput")
    bqkv = nc.dram_tensor("bqkv", [3 * CS, 1], F32, kind="ExternalInput")
    wproj = nc.dram_tensor("wproj", [CS, C], BF16, kind="ExternalInput")
    outT = nc.dram_tensor("outT", [C, BT], BF16, kind="ExternalOutput")

    with tile.TileContext(nc) as tc:
        _emit(nc, tc, xT.ap(), wqkv.ap(), bqkv.ap(), wproj.ap(), outT.ap(),
              repeat=repeat)
    nc.compile()
    _built[key] = nc
    return nc


def _emit(nc, tc, xT, wqkv, bqkv, wproj, outT, repeat=1, dbg=None):
    from contextlib import ExitStack
    ctx = ExitStack()
    with ctx:
        constp = ctx.enter_context(tc.tile_pool(name="const", bufs=1))
        xp = ctx.enter_context(tc.tile_pool(name="x", bufs=1))
        wp = ctx.enter_context(tc.tile_pool(name="w", bufs=1))
        qkvp = ctx.enter_context(tc.tile_pool(name="qkv", bufs=1))
        vnp = ctx.enter_context(tc.tile_pool(name="vnat", bufs=1))
        ppool = ctx.enter_context(tc.tile_pool(name="pp", bufs=4))
        ypool = ctx.enter_context(tc.tile_pool(name="yt", bufs=1))
        osp = ctx.enter_context(tc.tile_pool(name="ostage", bufs=4))
        rpool = ctx.enter_context(tc.tile_pool(name="rec", bufs=2))
        pss_bufs = PSS_BUFS or (2 if PSS_MODE == "shared" else 4)
        pss_banks = pss_bufs * (2 if PSS_MODE == "shared" else 1)
        psq_bufs = 8 - pss_banks - PSO_BUFS
        assert 1 <= psq_bufs <= 2, (pss_banks, PSO_BUFS)
        psS = ctx.enter_context(tc.tile_pool(name="psS", bufs=pss_bufs,
                                             space="PSUM"))
        psO = ctx.enter_context(tc.tile_pool(name="psO", bufs=PSO_BUFS,
                                             space="PSUM"))
        psQ = ctx.enter_context(tc.tile_pool(name="psQ", bufs=psq_bufs,
                                             space="PSUM"))

        # ---- constants / weights / inputs ----
        w_sb = wp.tile([128, NCT, 3, CS], BF16)
        nc.sync.dma_start(
            w_sb[:],
            wqkv.rearrange("(a p) (m c) -> p a m c", p=128, m=3))
        wp_sb = wp.tile([128, C], BF16)             # W_proj slice [CS=128, C]
        nc.sync.dma_start(wp_sb[:], wproj[:, :])
        bias_sb = wp.tile([128, 3], F32)
        nc.sync.dma_start(bias_sb[:],
                          bqkv.rearrange("(m p) o -> p (m o)", p=128))

        zbias = constp.tile([128, 1], F32)         # explicit exp bias=0:
        nc.gpsimd.memset(zbias[:], 0.0)            # a float bias would pull
        # in a const-AP DMA that queues behind all input DMAs

        ident = constp.tile([128, 128], BF16)      # for PE-mode transpose
        if TRP_MODE == "pe":
            from concourse.masks import make_identity
            make_identity(nc, ident[:])

        # causal mask for the 128x128 diagonal blocks of S^T: keep k <= q
        mask = constp.tile([128, KT], BF16)
        nc.gpsimd.memset(mask[:], 1.0)
        nc.gpsimd.affine_select(
            out=mask[:], in_=mask[:],
            compare_op=mybir.AluOpType.is_ge,
            fill=0.0, base=0, pattern=[[1, KT]],
            channel_multiplier=-1)

        # xT c-tiles, loaded per (token-chunk, c-tile) for early start
        x_sb = xp.tile([128, NCT, BT], BF16)
        XC = 512
        for nn_ in range(BT // XC):
            for a in range(NCT):
                nc.sync.dma_start(
                    x_sb[:, a, nn_ * XC:(nn_ + 1) * XC],
                    xT[a * 128:(a + 1) * 128, nn_ * XC:(nn_ + 1) * XC])

        # qkvT activations, [feat 128, tok] each; v produced transposed too
        q_sb = qkvp.tile([128, BT], BF16, tag="q")
        k_sb = qkvp.tile([128, BT], BF16, tag="k")
        vT_sb = qkvp.tile([128, BT], BF16, tag="vT")
        qkv_dst = [q_sb, k_sb, vT_sb]

        # v natural layout per (b, h, ktile): [tok 128, slot 128] with
        # cols [v(64) | ones | pad]: DMA-transpose needs 128-aligned dest
        # offsets, and the ones column makes the P@V matmul also emit the
        # softmax denominator (O' at psum partitions 0:64, denom at 64).
        vn_sb = vnp.tile([128, B, HPC, NKT, 128], BF16)
        nc.gpsimd.memset(vn_sb[:, :, :, :, 64:65], 1.0)

        yT_sb = ypool.tile([128, BT], BF16)         # per-core y^T slice
        if PHASES == "noattn":
            nc.gpsimd.memset(yT_sb[:], 0.0)        # proj needs a writer
        if PHASES == "attnonly":
            for t_ in qkv_dst:                     # attn needs writers
                nc.gpsimd.memset(t_[:], 0.0)

        def emit_qkv_group(b, n, m):
            if PHASES == "attnonly":
                return
            tb = b * T
            ps = psQ.tile([128, QB], F32, tag="psQ", name="qkvps")
            for a in range(NCT):
                nc.tensor.matmul(
                    ps[:], w_sb[:, a, m, :],
                    x_sb[:, a, tb + n * QB: tb + (n + 1) * QB],
                    start=(a == 0), stop=(a == NCT - 1))
            if QKV_EVAC == "scalar":
                nc.scalar.add(
                    qkv_dst[m][:, tb + n * QB: tb + (n + 1) * QB],
                    ps[:], bias_sb[:, m:m + 1])
            else:
                nc.vector.tensor_scalar_add(
                    qkv_dst[m][:, tb + n * QB: tb + (n + 1) * QB],
                    ps[:], bias_sb[:, m:m + 1])

        def emit_trp(b, i):
            tb = b * T
            if TRP_MODE == "pe":
                # PE transpose-mode; one [128,128] covers both heads.
                trp = psQ.tile([128, KT], BF16, tag="psQ", name="trp")
                nc.tensor.transpose(
                    trp[:], vT_sb[:, tb + i * KT: tb + (i + 1) * KT],
                    ident[:])
                for h in range(HPC):
                    nc.vector.tensor_copy(vn_sb[:, b, h, i, 0:64],
                                          trp[:, h * 64:(h + 1) * 64])
            else:
                # DMA xbar transposes, alone on the scalar-HWDGE queue
                # (mixing with copies on the same queues corrupted).
                for h in range(HPC):
                    nc.scalar.dma_start_transpose(
                        vn_sb[:, b, h, i, 0:64],
                        vT_sb[h * 64:(h + 1) * 64,
                              tb + i * KT: tb + (i + 1) * KT])

        def emit_attn_block(b, j):
            if PHASES == "noattn":
                return
            tb = b * T
            # O' accumulators: O' at partitions 0:64, denom at 64.
            ops = [psO.tile([65, QB], F32, tag="psO", name=f"op{h}")
                   for h in range(HPC)]
            nkt_j = 4 * (j + 1)
            for i in range(nkt_j):
                c0 = 0 if i < 4 * j else KT * (i - 4 * j)
                w = QB - c0
                # both heads' S^T (row-packed: h0 on array rows 0:64,
                # h1 on 64:128), either one 2-bank tile or two 1-bank
                if PSS_MODE == "shared":
                    st = psS.tile([128, 2 * QB], F32, tag="psS", name="s")
                    s_of = [(st, c0), (st, QB + c0)]
                else:
                    s_of = [(psS.tile([128, QB], F32, tag="psS", name="s"),
                             c0) for _ in range(HPC)]
                p = ppool.tile([128, 2 * QB], BF16, tag="pp", name="pp")
                for h in range(HPC):
                    hs = h * 64
                    st, of = s_of[h]
                    nc.tensor.matmul(
                        st[:, of: of + w],
                        k_sb[hs:hs + 64, tb + i * KT: tb + (i + 1) * KT],
                        q_sb[hs:hs + 64,
                             tb + j * QB + c0: tb + (j + 1) * QB],
                        start=True, stop=True)
                for h in range(HPC):
                    st, of = s_of[h]
                    nc.scalar.activation(
                        p[:, h * QB + c0: (h + 1) * QB],
                        st[:, of: of + w],
                        mybir.ActivationFunctionType.Exp,
                        bias=zbias[:, 0:1], scale=SCALE)
                if i >= 4 * j:
                    # zero the strict lower triangle of the 128x128
                    # diagonal block (causal mask) via mul with the
                    # precomputed triangle; split PV so its unmasked
                    # columns don't wait.
                    for h in range(HPC):
                        po = h * QB + c0
                        nc.vector.tensor_mul(
                            p[:, po:po + KT], p[:, po:po + KT], mask[:])
                    for h in range(HPC):
                        po = h * QB
                        if w > KT:
                            nc.tensor.matmul(
                                ops[h][0:65, c0 + KT:QB],
                                vn_sb[:, b, h, i, 0:65],
                                p[:, po + c0 + KT: po + QB],
                                start=(i == 0), stop=False)
                        nc.tensor.matmul(
                            ops[h][0:65, c0:c0 + KT],
                            vn_sb[:, b, h, i, 0:65],
                            p[:, po + c0: po + c0 + KT],
                            start=False, stop=(i == nkt_j - 1))
                else:
                    for h in range(HPC):
                        nc.tensor.matmul(
                            ops[h][0:65, c0:QB],
                            vn_sb[:, b, h, i, 0:65],
                            p[:, h * QB + c0: (h + 1) * QB],
                            start=(i == 0), stop=(i == nkt_j - 1))

            # normalize: y^T[:, block] = O' / denom. DVE lanes are
            # partition-rigid, so h=1's rows are produced at partitions
            # 0:64 and relocated to 64:128 by GpSimd.
            for h in range(HPC):
                rec = rpool.tile([65, QB], F32, tag="rec", name="rec")
                rec0 = rpool.tile([1, QB], F32, tag="rec0", name="rec0")
                rb = rpool.tile([64, QB], F32, tag="rb", name="rb")
                nc.vector.reciprocal(rec[64:65, :], ops[h][64:65, :])
                # partition_broadcast ucode reads absolute partition 0,
                # so hop the row down first (GpSimd is partition-flexible)
                nc.gpsimd.tensor_copy(rec0[0:1, :], rec[64:65, :])
                nc.gpsimd.partition_broadcast(rb[0:64, :], rec0[0:1, :])
                if h == 0:
                    nc.vector.tensor_mul(
                        yT_sb[0:64, tb + j * QB: tb + (j + 1) * QB],
                        ops[h][0:64, :], rb[0:64, :])
                else:
                    ytmp = rpool.tile([64, QB], BF16, tag="ytmp",
                                      name="ytmp")
                    nc.vector.tensor_mul(
                        ytmp[0:64, :], ops[h][0:64, :], rb[0:64, :])
                    nc.gpsimd.tensor_copy(
                        yT_sb[64:128, tb + j * QB: tb + (j + 1) * QB],
                        ytmp[0:64, :])

        def emit_proj(b, j):
            if PHASES == "attnonly":
                return
            tb = b * T
            for oc2 in range(NCT // 2):
                ost = osp.tile([128, 2 * QB], BF16, tag="ostage", name="ost")
                for k in range(2):
                    oc = oc2 * 2 + k
                    po = psQ.tile([128, QB], F32, tag="psQ", name="po")
                    nc.tensor.matmul(
                        po[:], wp_sb[:, oc * 128:(oc + 1) * 128],
                        yT_sb[:, tb + j * QB: tb + (j + 1) * QB],
                        start=True, stop=True)
                    nc.vector.tensor_copy(ost[:, k * QB:(k + 1) * QB], po[:])
                eng = (nc.scalar if OUT_SPLIT and oc2 % 2 else nc.sync)
                eng.dma_start(
                    outT[oc2 * 256:(oc2 + 1) * 256,
                         tb + j * QB: tb + (j + 1) * QB]
                    .rearrange("(t p) q -> p t q", t=2),
                    ost[:].rearrange("p (t q) -> p t q", t=2))

        # Wavefront emission: attention block j needs only token blocks
        # <= j of q/k/v, so it starts as soon as its slice of qkv/vn is
        # ready; qkv(b=1) and proj are woven in as PE/DVE filler for the
        # ACT-bound attention chain.
        for _rep in range(repeat):
            if SCHEDULE == "dual":
                # two independent attention streams (b=0, b=1) alternate
                # so one stream's normalize tail hides under the other's
                # i-loop; needs PSO_BUFS=4 (two blocks' accumulators live)
                for m in range(3):
                    emit_qkv_group(0, 0, m)
                for i in range(4):
                    emit_trp(0, i)
                emit_attn_block(0, 0)
                for n in range(1, NB):
                    for m in range(3):
                        emit_qkv_group(0, n, m)
                for i in range(4, NKT):
                    emit_trp(0, i)
                for n in range(NB):
                    for m in range(3):
                        emit_qkv_group(1, n, m)
                for i in range(4):
                    emit_trp(1, i)
                emit_attn_block(0, 1)
                emit_attn_block(1, 0)
                emit_proj(0, 0)
                for i in range(4, NKT):
                    emit_trp(1, i)
                emit_attn_block(0, 2)
                emit_attn_block(1, 1)
                emit_proj(0, 1)
                emit_attn_block(0, 3)
                emit_attn_block(1, 2)
                emit_proj(0, 2)
                emit_proj(1, 0)
                emit_attn_block(1, 3)
                emit_proj(0, 3)
                emit_proj(1, 1)
                emit_proj(1, 2)
                emit_proj(1, 3)
            else:
                for m in range(3):
                    emit_qkv_group(0, 0, m)
                for i in range(4):
                    emit_trp(0, i)
                emit_attn_block(0, 0)
                for n in range(1, NB):
                    for m in range(3):
                        emit_qkv_group(0, n, m)
                for i in range(4, NKT):
                    emit_trp(0, i)
                for j in range(1, NB):
                    emit_attn_block(0, j)
                    for m in range(3):
                        emit_qkv_group(1, j - 1, m)
                    emit_proj(0, j - 1)
                for i in range(4):
                    emit_trp(1, i)
                emit_attn_block(1, 0)
                for m in range(3):
                    emit_qkv_group(1, NB - 1, m)
                emit_proj(0, NB - 1)
                for j in range(1, NB):
                    for i in range(4 * j, 4 * j + 4):
                        emit_trp(1, i)
                    emit_attn_block(1, j)
                    emit_proj(1, j - 1)
                emit_proj(1, NB - 1)

        if dbg is not None:
            nc.sync.dma_start(dbg["q"].ap(), q_sb[:])
            nc.sync.dma_start(dbg["k"].ap(), k_sb[:])
            nc.sync.dma_start(dbg["vT"].ap(), vT_sb[:])
            nc.sync.dma_start(
                dbg["vn"].ap(),
                vn_sb[:].rearrange("p a b c d -> p (a b c d)"))
            nc.sync.dma_start(dbg["y"].ap(), yT_sb[:])
            nc.sync.dma_start(dbg["mask"].ap(), mask[:])


def _host_inputs(x, W_attn, b_attn):
    bf = ml_dtypes.bfloat16
    xTh = np.ascontiguousarray(
        x.reshape(BT, C).T.astype(bf))
    in_maps = []
    for c in range(NCORES):
        lo = c * CS
        wq = W_attn[:, lo:lo + CS]
        wk = W_attn[:, C + lo: C + lo + CS]
        wv = W_attn[:, 2 * C + lo: 2 * C + lo + CS]
        wqkv = np.ascontiguousarray(
            np.concatenate([wq, wk, wv], axis=1).astype(bf))
        bq = np.concatenate([b_attn[lo:lo + CS],
                             b_attn[C + lo: C + lo + CS],
                             b_attn[2 * C + lo: 2 * C + lo + CS]])
        bqkvh = np.ascontiguousarray(
            bq.reshape(3 * CS, 1).astype(np.float32))
        in_maps.append({"xT": xTh, "wqkv": wqkv, "bqkv": bqkvh})
    return in_maps


def kernel(x, W_attn, b_attn, W_proj, b_proj):
    x = np.asarray(x, np.float32)
    W_attn = np.asarray(W_attn, np.float32)
    b_attn = np.asarray(b_attn, np.float32)
    W_proj = np.asarray(W_proj, np.float32)
    b_proj = np.asarray(b_proj, np.float32)

    nc = _build()
    in_maps = _host_inputs(x, W_attn, b_attn)
    bf = ml_dtypes.bfloat16
    for c in range(NCORES):
        in_maps[c]["wproj"] = np.ascontiguousarray(
            W_proj[c * CS:(c + 1) * CS, :].astype(bf))

    res = bass_utils.run_bass_kernel_spmd(
        nc, in_maps, core_ids=list(range(NCORES)))
    acc = np.zeros((C, BT), np.float64)
    for c in range(NCORES):
        acc += res.results[c]["outT"].astype(np.float64)
    out = acc.T.astype(np.float32) + b_proj[None, :]
    return out.reshape(B, T, C)


# revision 35
# speedup vs baseline: 1.1470x; 1.1470x over previous
"""Causal self-attention (B=2, T=2048, C=1024, H=16) on 8 Trainium2 cores.

Sharding: tensor-parallel over heads (2 heads/core). Each core computes
q/k/v for its heads, causal attention, and its slice of the c_proj
contraction; the host sums the 8 partial projection outputs and adds
b_proj.

Device-side layout keeps activations transposed ([feat, tok]) so no
transposes of x/q/k are needed; v is transposed on-chip via DMA-xbar.
Softmax runs over the partition axis of S^T: the denominator comes for
free from a ones-column appended to v in the P@V matmul.

v2 engine budget: ACT runs only exp (+qkv bias evac); DVE does all
PSUM evacuation, the causal mask (mul by a precomputed triangle), and
approx reciprocals; GpSimd only does the tiny normalize hops; both
heads' S matmuls row-pack into one 2-bank PSUM tile so exp covers both
heads in (mostly) one instruction.
"""

import sys

try:
    import concourse  # noqa: F401
except ImportError:
    sys.path.insert(0, "/opt/trn_rl_repo")

import numpy as np
import ml_dtypes

import concourse.bacc as bacc
import concourse.mybir as mybir
import concourse.tile as tile
from concourse import bass_utils

B, T, C, H, NCORES = 2, 2048, 1024, 16, 8
BT = B * T                  # 4096 tokens total
HPC = H // NCORES           # 2 heads per core
D = C // H                  # 64 head dim
CS = HPC * D                # 128 per-core feature slice
QB = 512                    # q block (free dim per matmul)
KT = 128                    # k tile (partition dim of S^T)
NB = T // QB                # 4 q-blocks per batch
NKT = T // KT               # 16 k-tiles per batch
NCT = C // 128              # 8 contraction tiles over C
BF16 = mybir.dt.bfloat16
F32 = mybir.dt.float32
SCALE = 1.0 / np.sqrt(D)

_built = {}

# ---- tuning knobs (defaults = shipping config) ----
QKV_EVAC = "vector"      # "scalar" (ACT) | "vector" (DVE)
PSO_BUFS = 4             # psO bufs (4 = two blocks' accumulators live)
PSS_MODE = "perhead"     # "shared" [128,1024]x2 | "perhead" [128,512]xN
PSS_BUFS = 2             # 0 = auto (shared:2 / perhead:4)
PHASES = "all"           # "all" | "noattn" | "attnonly" (timing ablations)
TRP_MODE = "pe"          # "xbar" DMA transpose | "pe" PE transpose+DVE copy
SCHEDULE = "dual"        # "wave" | "dual" (interleave both batches' blocks)
OUT_SPLIT = False        # split output stores across both HWDGE rings
PV_SPLIT = True          # split diag PV so unmasked cols skip the mask dep


def _build(repeat=1):
    key = ("nc", repeat, QKV_EVAC, PSO_BUFS, PSS_MODE, PSS_BUFS, PHASES,
           TRP_MODE, SCHEDULE, OUT_SPLIT, PV_SPLIT)
    if key in _built:
        return _built[key]

    nc = bacc.Bacc("TRN2", target_bir_lowering=False, debug=False,
                   num_devices=NCORES)
    xT = nc.dram_tensor("xT", [C, BT], BF16, kind="ExternalInput")
    wqkv = nc.dram_tensor("wqkv", [C, 3 * CS], BF16, kind="ExternalInput")
    bqkv = nc.dram_tensor("bqkv", [3 * CS, 1], F32, kind="ExternalInput")
    wproj = nc.dram_tensor("wproj", [CS, C], BF16, kind="ExternalInput")
    outT = nc.dram_tensor("outT", [C, BT], BF16, kind="ExternalOutput")

    with tile.TileContext(nc) as tc:
        _emit(nc, tc, xT.ap(), wqkv.ap(), bqkv.ap(), wproj.ap(), outT.ap(),
              repeat=repeat)
    nc.compile()
    _built[key] = nc
    return nc


def _emit(nc, tc, xT, wqkv, bqkv, wproj, outT, repeat=1, dbg=None):
    from contextlib import ExitStack
    ctx = ExitStack()
    with ctx:
        constp = ctx.enter_context(tc.tile_pool(name="const", bufs=1))
        xp = ctx.enter_context(tc.tile_pool(name="x", bufs=1))
        wp = ctx.enter_context(tc.tile_pool(name="w", bufs=1))
        qkvp = ctx.enter_context(tc.tile_pool(name="qkv", bufs=1))
        vnp = ctx.enter_context(tc.tile_pool(name="vnat", bufs=1))
        ppool = ctx.enter_context(tc.tile_pool(name="pp", bufs=4))
        ypool = ctx.enter_context(tc.tile_pool(name="yt", bufs=1))
        osp = ctx.enter_context(tc.tile_pool(name="ostage", bufs=4))
        rpool = ctx.enter_context(tc.tile_pool(name="rec", bufs=2))
        pss_bufs = PSS_BUFS or (2 if PSS_MODE == "shared" else 4)
        pss_banks = pss_bufs * (2 if PSS_MODE == "shared" else 1)
        psq_bufs = 8 - pss_banks - PSO_BUFS
        assert 1 <= psq_bufs <= 2, (pss_banks, PSO_BUFS)
        psS = ctx.enter_context(tc.tile_pool(name="psS", bufs=pss_bufs,
                                             space="PSUM"))
        psO = ctx.enter_context(tc.tile_pool(name="psO", bufs=PSO_BUFS,
                                             space="PSUM"))
        psQ = ctx.enter_context(tc.tile_pool(name="psQ", bufs=psq_bufs,
                                             space="PSUM"))

        # ---- constants / weights / inputs ----
        w_sb = wp.tile([128, NCT, 3, CS], BF16)
        nc.sync.dma_start(
            w_sb[:],
            wqkv.rearrange("(a p) (m c) -> p a m c", p=128, m=3))
        wp_sb = wp.tile([128, C], BF16)             # W_proj slice [CS=128, C]
        nc.sync.dma_start(wp_sb[:], wproj[:, :])
        bias_sb = wp.tile([128, 3], F32)
        nc.sync.dma_start(bias_sb[:],
                          bqkv.rearrange("(m p) o -> p (m o)", p=128))

        zbias = constp.tile([128, 1], F32)         # explicit exp bias=0:
        nc.gpsimd.memset(zbias[:], 0.0)            # a float bias would pull
        # in a const-AP DMA that queues behind all input DMAs

        ident = constp.tile([128, 128], BF16)      # for PE-mode transpose
        if TRP_MODE == "pe":
            from concourse.masks import make_identity
            make_identity(nc, ident[:])

        # causal mask for the 128x128 diagonal blocks of S^T: keep k <= q
        mask = constp.tile([128, KT], BF16)
        nc.gpsimd.memset(mask[:], 1.0)
        nc.gpsimd.affine_select(
            out=mask[:], in_=mask[:],
            compare_op=mybir.AluOpType.is_ge,
            fill=0.0, base=0, pattern=[[1, KT]],
            channel_multiplier=-1)

        # xT c-tiles, loaded per (token-chunk, c-tile) for early start
        x_sb = xp.tile([128, NCT, BT], BF16)
        XC = 512
        for nn_ in range(BT // XC):
            for a in range(NCT):
                nc.sync.dma_start(
                    x_sb[:, a, nn_ * XC:(nn_ + 1) * XC],
                    xT[a * 128:(a + 1) * 128, nn_ * XC:(nn_ + 1) * XC])

        # qkvT activations, [feat 128, tok] each; v produced transposed too
        q_sb = qkvp.tile([128, BT], BF16, tag="q")
        k_sb = qkvp.tile([128, BT], BF16, tag="k")
        vT_sb = qkvp.tile([128, BT], BF16, tag="vT")
        qkv_dst = [q_sb, k_sb, vT_sb]

        # v natural layout per (b, h, ktile): [tok 128, slot 128] with
        # cols [v(64) | ones | pad]: DMA-transpose needs 128-aligned dest
        # offsets, and the ones column makes the P@V matmul also emit the
        # softmax denominator (O' at psum partitions 0:64, denom at 64).
        vn_sb = vnp.tile([128, B, HPC, NKT, 128], BF16)
        nc.gpsimd.memset(vn_sb[:, :, :, :, 64:65], 1.0)

        yT_sb = ypool.tile([128, BT], BF16)         # per-core y^T slice
        if PHASES == "noattn":
            nc.gpsimd.memset(yT_sb[:], 0.0)        # proj needs a writer
        if PHASES == "attnonly":
            for t_ in qkv_dst:                     # attn needs writers
                nc.gpsimd.memset(t_[:], 0.0)

        def emit_qkv_group(b, n, m):
            if PHASES == "attnonly":
                return
            tb = b * T
            ps = psQ.tile([128, QB], F32, tag="psQ", name="qkvps")
            for a in range(NCT):
                nc.tensor.matmul(
                    ps[:], w_sb[:, a, m, :],
                    x_sb[:, a, tb + n * QB: tb + (n + 1) * QB],
                    start=(a == 0), stop=(a == NCT - 1))
            if QKV_EVAC == "scalar":
                nc.scalar.add(
                    qkv_dst[m][:, tb + n * QB: tb + (n + 1) * QB],
                    ps[:], bias_sb[:, m:m + 1])
            else:
                nc.vector.tensor_scalar_add(
                    qkv_dst[m][:, tb + n * QB: tb + (n + 1) * QB],
                    ps[:], bias_sb[:, m:m + 1])

        def emit_trp(b, i):
            tb = b * T
            if TRP_MODE == "pe":
                # PE transpose-mode; one [128,128] covers both heads.
                trp = psQ.tile([128, KT], BF16, tag="psQ", name="trp")
                nc.tensor.transpose(
                    trp[:], vT_sb[:, tb + i * KT: tb + (i + 1) * KT],
                    ident[:])
                for h in range(HPC):
                    nc.vector.tensor_copy(vn_sb[:, b, h, i, 0:64],
                                          trp[:, h * 64:(h + 1) * 64])
            else:
                # DMA xbar transposes, alone on the scalar-HWDGE queue
                # (mixing with copies on the same queues corrupted).
                for h in range(HPC):
                    nc.scalar.dma_start_transpose(
                        vn_sb[:, b, h, i, 0:64],
                        vT_sb[h * 64:(h + 1) * 64,
                              tb + i * KT: tb + (i + 1) * KT])

        def emit_attn_block(b, j):
            if PHASES == "noattn":
                return
            tb = b * T
            # O' accumulators: O' at partitions 0:64, denom at 64.
            ops = [psO.tile([65, QB], F32, tag="psO", name=f"op{h}")
                   for h in range(HPC)]
            nkt_j = 4 * (j + 1)
            for i in range(nkt_j):
                c0 = 0 if i < 4 * j else KT * (i - 4 * j)
                w = QB - c0
                # both heads' S^T (row-packed: h0 on array rows 0:64,
                # h1 on 64:128), either one 2-bank tile or two 1-bank
                if PSS_MODE == "shared":
                    st = psS.tile([128, 2 * QB], F32, tag="psS", name="s")
                    s_of = [(st, c0), (st, QB + c0)]
                else:
                    s_of = [(psS.tile([128, QB], F32, tag="psS", name="s"),
                             c0) for _ in range(HPC)]
                p = ppool.tile([128, 2 * QB], BF16, tag="pp", name="pp")
                for h in range(HPC):
                    hs = h * 64
                    st, of = s_of[h]
                    nc.tensor.matmul(
                        st[:, of: of + w],
                        k_sb[hs:hs + 64, tb + i * KT: tb + (i + 1) * KT],
                        q_sb[hs:hs + 64,
                             tb + j * QB + c0: tb + (j + 1) * QB],
                        start=True, stop=True)
                for h in range(HPC):
                    st, of = s_of[h]
                    nc.scalar.activation(
                        p[:, h * QB + c0: (h + 1) * QB],
                        st[:, of: of + w],
                        mybir.ActivationFunctionType.Exp,
                        bias=zbias[:, 0:1], scale=SCALE)
                if i >= 4 * j:
                    # zero the strict lower triangle of the 128x128
                    # diagonal block (causal mask) via mul with the
                    # precomputed triangle; split PV so its unmasked
                    # columns don't wait.
                    for h in range(HPC):
                        po = h * QB + c0
                        nc.vector.tensor_mul(
                            p[:, po:po + KT], p[:, po:po + KT], mask[:])
                    for h in range(HPC):
                        po = h * QB
                        if PV_SPLIT and w > KT:
                            nc.tensor.matmul(
                                ops[h][0:65, c0 + KT:QB],
                                vn_sb[:, b, h, i, 0:65],
                                p[:, po + c0 + KT: po + QB],
                                start=(i == 0), stop=False)
                            nc.tensor.matmul(
                                ops[h][0:65, c0:c0 + KT],
                                vn_sb[:, b, h, i, 0:65],
                                p[:, po + c0: po + c0 + KT],
                                start=False, stop=(i == nkt_j - 1))
                        else:
                            nc.tensor.matmul(
                                ops[h][0:65, c0:QB],
                                vn_sb[:, b, h, i, 0:65],
                                p[:, po + c0: po + QB],
                                start=(i == 0), stop=(i == nkt_j - 1))
                else:
                    for h in range(HPC):
                        nc.tensor.matmul(
                            ops[h][0:65, c0:QB],
                            vn_sb[:, b, h, i, 0:65],
                            p[:, h * QB + c0: (h + 1) * QB],
                            start=(i == 0), stop=(i == nkt_j - 1))

            # normalize: y^T[:, block] = O' / denom. DVE lanes are
            # partition-rigid, so h=1's rows are produced at partitions
            # 0:64 and relocated to 64:128 by GpSimd.
            for h in range(HPC):
                rec = rpool.tile([65, QB], F32, tag="rec", name="rec")
                rec0 = rpool.tile([1, QB], F32, tag="rec0", name="rec0")
                rb = rpool.tile([64, QB], F32, tag="rb", name="rb")
                nc.vector.reciprocal(rec[64:65, :], ops[h][64:65, :])
                # partition_broadcast ucode reads absolute partition 0,
                # so hop the row down first (GpSimd is partition-flexible)
                nc.gpsimd.tensor_copy(rec0[0:1, :], rec[64:65, :])
                nc.gpsimd.partition_broadcast(rb[0:64, :], rec0[0:1, :])
                if h == 0:
                    nc.vector.tensor_mul(
                        yT_sb[0:64, tb + j * QB: tb + (j + 1) * QB],
                        ops[h][0:64, :], rb[0:64, :])
                else:
                    ytmp = rpool.tile([64, QB], BF16, tag="ytmp",
                                      name="ytmp")
                    nc.vector.tensor_mul(
                        ytmp[0:64, :], ops[h][0:64, :], rb[0:64, :])
                    nc.gpsimd.tensor_copy(
                        yT_sb[64:128, tb + j * QB: tb + (j + 1) * QB],
                        ytmp[0:64, :])

        def emit_proj(b, j):
            if PHASES == "attnonly":
                return
            tb = b * T
            for oc2 in range(NCT // 2):
                ost = osp.tile([128, 2 * QB], BF16, tag="ostage", name="ost")
                for k in range(2):
                    oc = oc2 * 2 + k
                    po = psQ.tile([128, QB], F32, tag="psQ", name="po")
                    nc.tensor.matmul(
                        po[:], wp_sb[:, oc * 128:(oc + 1) * 128],
                        yT_sb[:, tb + j * QB: tb + (j + 1) * QB],
                        start=True, stop=True)
                    nc.vector.tensor_copy(ost[:, k * QB:(k + 1) * QB], po[:])
                eng = (nc.scalar if OUT_SPLIT and oc2 % 2 else nc.sync)
                eng.dma_start(
                    outT[oc2 * 256:(oc2 + 1) * 256,
                         tb + j * QB: tb + (j + 1) * QB]
                    .rearrange("(t p) q -> p t q", t=2),
                    ost[:].rearrange("p (t q) -> p t q", t=2))

        # Wavefront emission: attention block j needs only token blocks
        # <= j of q/k/v, so it starts as soon as its slice of qkv/vn is
        # ready; qkv(b=1) and proj are woven in as PE/DVE filler for the
        # ACT-bound attention chain.
        for _rep in range(repeat):
            if SCHEDULE == "dual":
                # two independent attention streams (b=0, b=1) alternate
                # so one stream's normalize tail hides under the other's
                # i-loop; needs PSO_BUFS=4 (two blocks' accumulators live)
                for m in range(3):
                    emit_qkv_group(0, 0, m)
                for i in range(4):
                    emit_trp(0, i)
                emit_attn_block(0, 0)
                for n in range(1, NB):
                    for m in range(3):
                        emit_qkv_group(0, n, m)
                for i in range(4, NKT):
                    emit_trp(0, i)
                for n in range(NB):
                    for m in range(3):
                        emit_qkv_group(1, n, m)
                for i in range(4):
                    emit_trp(1, i)
                emit_attn_block(0, 1)
                emit_attn_block(1, 0)
                emit_proj(0, 0)
                for i in range(4, NKT):
                    emit_trp(1, i)
                emit_attn_block(0, 2)
                emit_attn_block(1, 1)
                emit_proj(0, 1)
                emit_attn_block(0, 3)
                emit_attn_block(1, 2)
                emit_proj(0, 2)
                emit_proj(1, 0)
                emit_attn_block(1, 3)
                emit_proj(0, 3)
                emit_proj(1, 1)
                emit_proj(1, 2)
                emit_proj(1, 3)
            else:
                for m in range(3):
                    emit_qkv_group(0, 0, m)
                for i in range(4):
                    emit_trp(0, i)
                emit_attn_block(0, 0)
                for n in range(1, NB):
                    for m in range(3):
                        emit_qkv_group(0, n, m)
                for i in range(4, NKT):
                    emit_trp(0, i)
                for j in range(1, NB):
                    emit_attn_block(0, j)
                    for m in range(3):
                        emit_qkv_group(1, j - 1, m)
                    emit_proj(0, j - 1)
                for i in range(4):
                    emit_trp(1, i)
                emit_attn_block(1, 0)
                for m in range(3):
                    emit_qkv_group(1, NB - 1, m)
                emit_proj(0, NB - 1)
                for j in range(1, NB):
                    for i in range(4 * j, 4 * j + 4):
                        emit_trp(1, i)
                    emit_attn_block(1, j)
                    emit_proj(1, j - 1)
                emit_proj(1, NB - 1)

        if dbg is not None:
            nc.sync.dma_start(dbg["q"].ap(), q_sb[:])
            nc.sync.dma_start(dbg["k"].ap(), k_sb[:])
            nc.sync.dma_start(dbg["vT"].ap(), vT_sb[:])
            nc.sync.dma_start(
                dbg["vn"].ap(),
                vn_sb[:].rearrange("p a b c d -> p (a b c d)"))
            nc.sync.dma_start(dbg["y"].ap(), yT_sb[:])
            nc.sync.dma_start(dbg["mask"].ap(), mask[:])


def _host_inputs(x, W_attn, b_attn):
    bf = ml_dtypes.bfloat16
    xTh = np.ascontiguousarray(
        x.reshape(BT, C).T.astype(bf))
    in_maps = []
    for c in range(NCORES):
        lo = c * CS
        wq = W_attn[:, lo:lo + CS]
        wk = W_attn[:, C + lo: C + lo + CS]
        wv = W_attn[:, 2 * C + lo: 2 * C + lo + CS]
        wqkv = np.ascontiguousarray(
            np.concatenate([wq, wk, wv], axis=1).astype(bf))
        bq = np.concatenate([b_attn[lo:lo + CS],
                             b_attn[C + lo: C + lo + CS],
                             b_attn[2 * C + lo: 2 * C + lo + CS]])
        bqkvh = np.ascontiguousarray(
            bq.reshape(3 * CS, 1).astype(np.float32))
        in_maps.append({"xT": xTh, "wqkv": wqkv, "bqkv": bqkvh})
    return in_maps


def kernel(x, W_attn, b_attn, W_proj, b_proj):
    x = np.asarray(x, np.float32)
    W_attn = np.asarray(W_attn, np.float32)
    b_attn = np.asarray(b_attn, np.float32)
    W_proj = np.asarray(W_proj, np.float32)
    b_proj = np.asarray(b_proj, np.float32)

    nc = _build()
    in_maps = _host_inputs(x, W_attn, b_attn)
    bf = ml_dtypes.bfloat16
    for c in range(NCORES):
        in_maps[c]["wproj"] = np.ascontiguousarray(
            W_proj[c * CS:(c + 1) * CS, :].astype(bf))

    res = bass_utils.run_bass_kernel_spmd(
        nc, in_maps, core_ids=list(range(NCORES)))
    acc = np.zeros((C, BT), np.float64)
    for c in range(NCORES):
        acc += res.results[c]["outT"].astype(np.float64)
    out = acc.T.astype(np.float32) + b_proj[None, :]
    return out.reshape(B, T, C)


# revision 41
# speedup vs baseline: 1.4354x; 1.2515x over previous
"""Causal self-attention (B=2, T=2048, C=1024, H=16) on 8 Trainium2 cores.

Sharding: tensor-parallel over heads (2 heads/core). Each core computes
q/k/v for its heads, causal attention, and its slice of the c_proj
contraction; the host sums the 8 partial projection outputs and adds
b_proj.

Device-side layout keeps activations transposed ([feat, tok]) so no
transposes of x/q/k are needed; v is transposed on-chip via DMA-xbar.
Softmax runs over the partition axis of S^T: the denominator comes for
free from a ones-column appended to v in the P@V matmul.

v2 engine budget: ACT runs only exp (+qkv bias evac); DVE does all
PSUM evacuation, the causal mask (mul by a precomputed triangle), and
approx reciprocals; GpSimd only does the tiny normalize hops; both
heads' S matmuls row-pack into one 2-bank PSUM tile so exp covers both
heads in (mostly) one instruction.
"""

import sys

try:
    import concourse  # noqa: F401
except ImportError:
    sys.path.insert(0, "/opt/trn_rl_repo")

import numpy as np
import ml_dtypes

import concourse.bacc as bacc
import concourse.mybir as mybir
import concourse.tile as tile
from concourse import bass_utils

B, T, C, H, NCORES = 2, 2048, 1024, 16, 8
BT = B * T                  # 4096 tokens total
HPC = H // NCORES           # 2 heads per core
D = C // H                  # 64 head dim
CS = HPC * D                # 128 per-core feature slice
QB = 512                    # q block (free dim per matmul)
KT = 128                    # k tile (partition dim of S^T)
NB = T // QB                # 4 q-blocks per batch
NKT = T // KT               # 16 k-tiles per batch
NCT = C // 128              # 8 contraction tiles over C
BF16 = mybir.dt.bfloat16
F32 = mybir.dt.float32
SCALE = 1.0 / np.sqrt(D)

_built = {}

# ---- tuning knobs (defaults = shipping config) ----
QKV_EVAC = "vector"      # "scalar" (ACT) | "vector" (DVE)
PSO_BUFS = 3             # psO bufs (dual streams' accumulators)
PSS_MODE = "perhead"     # "shared" [128,1024]x2 | "perhead" [128,512]xN
PSS_BUFS = 3             # 0 = auto (shared:2 / perhead:4)
PHASES = "all"           # "all" | "noattn" | "attnonly" (timing ablations)
TRP_MODE = "pe"          # "xbar" DMA transpose | "pe" PE transpose+DVE copy
SCHEDULE = "dual"        # "wave" | "dual" (interleave both batches' blocks)
OUT_SPLIT = False        # split output stores across both HWDGE rings
PV_SPLIT = True          # split diag PV so unmasked cols skip the mask dep
TRP_POOL = "psQ"         # which pool tag stages PE-mode transposes


def _build(repeat=1):
    key = ("nc", repeat, QKV_EVAC, PSO_BUFS, PSS_MODE, PSS_BUFS, PHASES,
           TRP_MODE, SCHEDULE, OUT_SPLIT, PV_SPLIT, TRP_POOL)
    if key in _built:
        return _built[key]

    nc = bacc.Bacc("TRN2", target_bir_lowering=False, debug=False,
                   num_devices=NCORES)
    xT = nc.dram_tensor("xT", [C, BT], BF16, kind="ExternalInput")
    wqkv = nc.dram_tensor("wqkv", [C, 3 * CS], BF16, kind="ExternalInput")
    bqkv = nc.dram_tensor("bqkv", [3 * CS, 1], F32, kind="ExternalInput")
    wproj = nc.dram_tensor("wproj", [CS, C], BF16, kind="ExternalInput")
    outT = nc.dram_tensor("outT", [C, BT], BF16, kind="ExternalOutput")

    with tile.TileContext(nc) as tc:
        _emit(nc, tc, xT.ap(), wqkv.ap(), bqkv.ap(), wproj.ap(), outT.ap(),
              repeat=repeat)
    nc.compile()
    _built[key] = nc
    return nc


def _emit(nc, tc, xT, wqkv, bqkv, wproj, outT, repeat=1, dbg=None):
    from contextlib import ExitStack
    ctx = ExitStack()
    with ctx:
        constp = ctx.enter_context(tc.tile_pool(name="const", bufs=1))
        xp = ctx.enter_context(tc.tile_pool(name="x", bufs=1))
        wp = ctx.enter_context(tc.tile_pool(name="w", bufs=1))
        qkvp = ctx.enter_context(tc.tile_pool(name="qkv", bufs=1))
        vnp = ctx.enter_context(tc.tile_pool(name="vnat", bufs=1))
        ppool = ctx.enter_context(tc.tile_pool(name="pp", bufs=4))
        ypool = ctx.enter_context(tc.tile_pool(name="yt", bufs=1))
        osp = ctx.enter_context(tc.tile_pool(name="ostage", bufs=4))
        rpool = ctx.enter_context(tc.tile_pool(name="rec", bufs=2))
        pss_bufs = PSS_BUFS or (2 if PSS_MODE == "shared" else 4)
        pss_banks = pss_bufs * (2 if PSS_MODE == "shared" else 1)
        psq_bufs = 8 - pss_banks - PSO_BUFS
        assert 1 <= psq_bufs <= 2, (pss_banks, PSO_BUFS)
        psS = ctx.enter_context(tc.tile_pool(name="psS", bufs=pss_bufs,
                                             space="PSUM"))
        psO = ctx.enter_context(tc.tile_pool(name="psO", bufs=PSO_BUFS,
                                             space="PSUM"))
        psQ = ctx.enter_context(tc.tile_pool(name="psQ", bufs=psq_bufs,
                                             space="PSUM"))

        # ---- constants / weights / inputs ----
        w_sb = wp.tile([128, NCT, 3, CS], BF16)
        nc.sync.dma_start(
            w_sb[:],
            wqkv.rearrange("(a p) (m c) -> p a m c", p=128, m=3))
        wp_sb = wp.tile([128, C], BF16)             # W_proj slice [CS=128, C]
        nc.sync.dma_start(wp_sb[:], wproj[:, :])
        bias_sb = wp.tile([128, 3], F32)
        nc.sync.dma_start(bias_sb[:],
                          bqkv.rearrange("(m p) o -> p (m o)", p=128))

        zbias = constp.tile([128, 1], F32)         # explicit exp bias=0:
        nc.gpsimd.memset(zbias[:], 0.0)            # a float bias would pull
        # in a const-AP DMA that queues behind all input DMAs

        ident = constp.tile([128, 128], BF16)      # for PE-mode transpose
        if TRP_MODE == "pe":
            from concourse.masks import make_identity
            make_identity(nc, ident[:])

        # causal mask for the 128x128 diagonal blocks of S^T: keep k <= q
        mask = constp.tile([128, KT], BF16)
        nc.gpsimd.memset(mask[:], 1.0)
        nc.gpsimd.affine_select(
            out=mask[:], in_=mask[:],
            compare_op=mybir.AluOpType.is_ge,
            fill=0.0, base=0, pattern=[[1, KT]],
            channel_multiplier=-1)

        # xT c-tiles, loaded per (token-chunk, c-tile) for early start
        x_sb = xp.tile([128, NCT, BT], BF16)
        XC = 512
        for nn_ in range(BT // XC):
            for a in range(NCT):
                nc.sync.dma_start(
                    x_sb[:, a, nn_ * XC:(nn_ + 1) * XC],
                    xT[a * 128:(a + 1) * 128, nn_ * XC:(nn_ + 1) * XC])

        # qkvT activations, [feat 128, tok] each; v produced transposed too
        q_sb = qkvp.tile([128, BT], BF16, tag="q")
        k_sb = qkvp.tile([128, BT], BF16, tag="k")
        vT_sb = qkvp.tile([128, BT], BF16, tag="vT")
        qkv_dst = [q_sb, k_sb, vT_sb]

        # v natural layout per (b, h, ktile): [tok 128, slot 128] with
        # cols [v(64) | ones | pad]: DMA-transpose needs 128-aligned dest
        # offsets, and the ones column makes the P@V matmul also emit the
        # softmax denominator (O' at psum partitions 0:64, denom at 64).
        vn_sb = vnp.tile([128, B, HPC, NKT, 128], BF16)
        nc.gpsimd.memset(vn_sb[:, :, :, :, 64:65], 1.0)

        yT_sb = ypool.tile([128, BT], BF16)         # per-core y^T slice
        if PHASES == "noattn":
            nc.gpsimd.memset(yT_sb[:], 0.0)        # proj needs a writer
        if PHASES == "attnonly":
            for t_ in qkv_dst:                     # attn needs writers
                nc.gpsimd.memset(t_[:], 0.0)

        def emit_qkv_group(b, n, m):
            if PHASES == "attnonly":
                return
            tb = b * T
            ps = psQ.tile([128, QB], F32, tag="psQ", name="qkvps")
            for a in range(NCT):
                nc.tensor.matmul(
                    ps[:], w_sb[:, a, m, :],
                    x_sb[:, a, tb + n * QB: tb + (n + 1) * QB],
                    start=(a == 0), stop=(a == NCT - 1))
            if QKV_EVAC == "scalar":
                nc.scalar.add(
                    qkv_dst[m][:, tb + n * QB: tb + (n + 1) * QB],
                    ps[:], bias_sb[:, m:m + 1])
            else:
                nc.vector.tensor_scalar_add(
                    qkv_dst[m][:, tb + n * QB: tb + (n + 1) * QB],
                    ps[:], bias_sb[:, m:m + 1])

        def emit_trp(b, i):
            tb = b * T
            if TRP_MODE == "pe":
                # PE transpose-mode; one [128,128] covers both heads.
                pool, tag = ((psS, "psS") if TRP_POOL == "psS"
                             else (psQ, "psQ"))
                trp = pool.tile([128, KT], BF16, tag=tag, name="trp")
                nc.tensor.transpose(
                    trp[:], vT_sb[:, tb + i * KT: tb + (i + 1) * KT],
                    ident[:])
                for h in range(HPC):
                    nc.vector.tensor_copy(vn_sb[:, b, h, i, 0:64],
                                          trp[:, h * 64:(h + 1) * 64])
            else:
                # DMA xbar transposes, alone on the scalar-HWDGE queue
                # (mixing with copies on the same queues corrupted).
                for h in range(HPC):
                    nc.scalar.dma_start_transpose(
                        vn_sb[:, b, h, i, 0:64],
                        vT_sb[h * 64:(h + 1) * 64,
                              tb + i * KT: tb + (i + 1) * KT])

        def emit_attn_block(b, j):
            if PHASES == "noattn":
                return
            tb = b * T
            # O' accumulators: O' at partitions 0:64, denom at 64.
            ops = [psO.tile([65, QB], F32, tag="psO", name=f"op{h}")
                   for h in range(HPC)]
            nkt_j = 4 * (j + 1)
            for i in range(nkt_j):
                c0 = 0 if i < 4 * j else KT * (i - 4 * j)
                w = QB - c0
                # both heads' S^T (row-packed: h0 on array rows 0:64,
                # h1 on 64:128), either one 2-bank tile or two 1-bank
                if PSS_MODE == "shared":
                    st = psS.tile([128, 2 * QB], F32, tag="psS", name="s")
                    s_of = [(st, c0), (st, QB + c0)]
                else:
                    s_of = [(psS.tile([128, QB], F32, tag="psS", name="s"),
                             c0) for _ in range(HPC)]
                p = ppool.tile([128, 2 * QB], BF16, tag="pp", name="pp")
                for h in range(HPC):
                    hs = h * 64
                    st, of = s_of[h]
                    nc.tensor.matmul(
                        st[:, of: of + w],
                        k_sb[hs:hs + 64, tb + i * KT: tb + (i + 1) * KT],
                        q_sb[hs:hs + 64,
                             tb + j * QB + c0: tb + (j + 1) * QB],
                        start=True, stop=True)
                for h in range(HPC):
                    st, of = s_of[h]
                    nc.scalar.activation(
                        p[:, h * QB + c0: (h + 1) * QB],
                        st[:, of: of + w],
                        mybir.ActivationFunctionType.Exp,
                        bias=zbias[:, 0:1], scale=SCALE)
                if i >= 4 * j:
                    # zero the strict lower triangle of the 128x128
                    # diagonal block (causal mask) via mul with the
                    # precomputed triangle; split PV so its unmasked
                    # columns don't wait.
                    for h in range(HPC):
                        po = h * QB + c0
                        nc.vector.tensor_mul(
                            p[:, po:po + KT], p[:, po:po + KT], mask[:])
                    for h in range(HPC):
                        po = h * QB
                        if PV_SPLIT and w > KT:
                            nc.tensor.matmul(
                                ops[h][0:65, c0 + KT:QB],
                                vn_sb[:, b, h, i, 0:65],
                                p[:, po + c0 + KT: po + QB],
                                start=(i == 0), stop=False)
                            nc.tensor.matmul(
                                ops[h][0:65, c0:c0 + KT],
                                vn_sb[:, b, h, i, 0:65],
                                p[:, po + c0: po + c0 + KT],
                                start=False, stop=(i == nkt_j - 1))
                        else:
                            nc.tensor.matmul(
                                ops[h][0:65, c0:QB],
                                vn_sb[:, b, h, i, 0:65],
                                p[:, po + c0: po + QB],
                                start=(i == 0), stop=(i == nkt_j - 1))
                else:
                    for h in range(HPC):
                        nc.tensor.matmul(
                            ops[h][0:65, c0:QB],
                            vn_sb[:, b, h, i, 0:65],
                            p[:, h * QB + c0: (h + 1) * QB],
                            start=(i == 0), stop=(i == nkt_j - 1))

            # normalize: y^T[:, block] = O' / denom. DVE lanes are
            # partition-rigid, so h=1's rows are produced at partitions
            # 0:64 and relocated to 64:128 by GpSimd.
            for h in range(HPC):
                rec = rpool.tile([65, QB], F32, tag="rec", name="rec")
                rec0 = rpool.tile([1, QB], F32, tag="rec0", name="rec0")
                rb = rpool.tile([64, QB], F32, tag="rb", name="rb")
                nc.vector.reciprocal(rec[64:65, :], ops[h][64:65, :])
                # partition_broadcast ucode reads absolute partition 0,
                # so hop the row down first (GpSimd is partition-flexible)
                nc.gpsimd.tensor_copy(rec0[0:1, :], rec[64:65, :])
                nc.gpsimd.partition_broadcast(rb[0:64, :], rec0[0:1, :])
                if h == 0:
                    nc.vector.tensor_mul(
                        yT_sb[0:64, tb + j * QB: tb + (j + 1) * QB],
                        ops[h][0:64, :], rb[0:64, :])
                else:
                    ytmp = rpool.tile([64, QB], BF16, tag="ytmp",
                                      name="ytmp")
                    nc.vector.tensor_mul(
                        ytmp[0:64, :], ops[h][0:64, :], rb[0:64, :])
                    nc.gpsimd.tensor_copy(
                        yT_sb[64:128, tb + j * QB: tb + (j + 1) * QB],
                        ytmp[0:64, :])

        def emit_proj(b, j):
            if PHASES == "attnonly":
                return
            tb = b * T
            for oc2 in range(NCT // 2):
                ost = osp.tile([128, 2 * QB], BF16, tag="ostage", name="ost")
                for k in range(2):
                    oc = oc2 * 2 + k
                    po = psQ.tile([128, QB], F32, tag="psQ", name="po")
                    nc.tensor.matmul(
                        po[:], wp_sb[:, oc * 128:(oc + 1) * 128],
                        yT_sb[:, tb + j * QB: tb + (j + 1) * QB],
                        start=True, stop=True)
                    nc.vector.tensor_copy(ost[:, k * QB:(k + 1) * QB], po[:])
                eng = (nc.scalar if OUT_SPLIT and oc2 % 2 else nc.sync)
                eng.dma_start(
                    outT[oc2 * 256:(oc2 + 1) * 256,
                         tb + j * QB: tb + (j + 1) * QB]
                    .rearrange("(t p) q -> p t q", t=2),
                    ost[:].rearrange("p (t q) -> p t q", t=2))

        # Wavefront emission: attention block j needs only token blocks
        # <= j of q/k/v, so it starts as soon as its slice of qkv/vn is
        # ready; qkv(b=1) and proj are woven in as PE/DVE filler for the
        # ACT-bound attention chain.
        for _rep in range(repeat):
            if SCHEDULE in ("dual", "dual2"):
                # two independent attention streams (b=0, b=1) alternate
                # so one stream's normalize tail hides under the other's
                # i-loop; needs PSO_BUFS=4 (two blocks' accumulators live)
                for m in range(3):
                    emit_qkv_group(0, 0, m)
                for i in range(4):
                    emit_trp(0, i)
                emit_attn_block(0, 0)
                for n in range(1, NB):
                    for m in range(3):
                        emit_qkv_group(0, n, m)
                for i in range(4, NKT):
                    emit_trp(0, i)
                for n in range(NB):
                    for m in range(3):
                        emit_qkv_group(1, n, m)
                for i in range(4):
                    emit_trp(1, i)
                emit_attn_block(0, 1)
                emit_attn_block(1, 0)
                emit_proj(0, 0)
                for i in range(4, NKT):
                    emit_trp(1, i)
                emit_attn_block(0, 2)
                emit_attn_block(1, 1)
                emit_proj(0, 1)
                if SCHEDULE == "dual2":
                    # keep proj(1,*) one block behind its stream instead
                    # of bunching them serially at the tail
                    emit_proj(1, 0)
                    emit_attn_block(0, 3)
                    emit_attn_block(1, 2)
                    emit_proj(0, 2)
                    emit_proj(1, 1)
                    emit_attn_block(1, 3)
                    emit_proj(0, 3)
                    emit_proj(1, 2)
                    emit_proj(1, 3)
                else:
                    emit_attn_block(0, 3)
                    emit_attn_block(1, 2)
                    emit_proj(0, 2)
                    emit_proj(1, 0)
                    emit_attn_block(1, 3)
                    emit_proj(0, 3)
                    emit_proj(1, 1)
                    emit_proj(1, 2)
                    emit_proj(1, 3)
            else:
                for m in range(3):
                    emit_qkv_group(0, 0, m)
                for i in range(4):
                    emit_trp(0, i)
                emit_attn_block(0, 0)
                for n in range(1, NB):
                    for m in range(3):
                        emit_qkv_group(0, n, m)
                for i in range(4, NKT):
                    emit_trp(0, i)
                for j in range(1, NB):
                    emit_attn_block(0, j)
                    for m in range(3):
                        emit_qkv_group(1, j - 1, m)
                    emit_proj(0, j - 1)
                for i in range(4):
                    emit_trp(1, i)
                emit_attn_block(1, 0)
                for m in range(3):
                    emit_qkv_group(1, NB - 1, m)
                emit_proj(0, NB - 1)
                for j in range(1, NB):
                    for i in range(4 * j, 4 * j + 4):
                        emit_trp(1, i)
                    emit_attn_block(1, j)
                    emit_proj(1, j - 1)
                emit_proj(1, NB - 1)

        if dbg is not None:
            nc.sync.dma_start(dbg["q"].ap(), q_sb[:])
            nc.sync.dma_start(dbg["k"].ap(), k_sb[:])
            nc.sync.dma_start(dbg["vT"].ap(), vT_sb[:])
            nc.sync.dma_start(
                dbg["vn"].ap(),
                vn_sb[:].rearrange("p a b c d -> p (a b c d)"))
            nc.sync.dma_start(dbg["y"].ap(), yT_sb[:])
            nc.sync.dma_start(dbg["mask"].ap(), mask[:])


def _host_inputs(x, W_attn, b_attn):
    bf = ml_dtypes.bfloat16
    xTh = np.ascontiguousarray(
        x.reshape(BT, C).T.astype(bf))
    in_maps = []
    for c in range(NCORES):
        lo = c * CS
        wq = W_attn[:, lo:lo + CS]
        wk = W_attn[:, C + lo: C + lo + CS]
        wv = W_attn[:, 2 * C + lo: 2 * C + lo + CS]
        wqkv = np.ascontiguousarray(
            np.concatenate([wq, wk, wv], axis=1).astype(bf))
        bq = np.concatenate([b_attn[lo:lo + CS],
                             b_attn[C + lo: C + lo + CS],
                             b_attn[2 * C + lo: 2 * C + lo + CS]])
        bqkvh = np.ascontiguousarray(
            bq.reshape(3 * CS, 1).astype(np.float32))
        in_maps.append({"xT": xTh, "wqkv": wqkv, "bqkv": bqkvh})
    return in_maps


def kernel(x, W_attn, b_attn, W_proj, b_proj):
    x = np.asarray(x, np.float32)
    W_attn = np.asarray(W_attn, np.float32)
    b_attn = np.asarray(b_attn, np.float32)
    W_proj = np.asarray(W_proj, np.float32)
    b_proj = np.asarray(b_proj, np.float32)

    nc = _build()
    in_maps = _host_inputs(x, W_attn, b_attn)
    bf = ml_dtypes.bfloat16
    for c in range(NCORES):
        in_maps[c]["wproj"] = np.ascontiguousarray(
            W_proj[c * CS:(c + 1) * CS, :].astype(bf))

    res = bass_utils.run_bass_kernel_spmd(
        nc, in_maps, core_ids=list(range(NCORES)))
    acc = np.zeros((C, BT), np.float64)
    for c in range(NCORES):
        acc += res.results[c]["outT"].astype(np.float64)
    out = acc.T.astype(np.float32) + b_proj[None, :]
    return out.reshape(B, T, C)
